# revision 38
# baseline (speedup 1.0000x reference)
"""NetVLAD layer on 8 Trainium2 NeuronCores (Bass/Tile), v3-final.

Problem: descriptors [B=16, D=512, N=4096] f32, W [K=64, D], b [K],
centers [D, K].
  scores = softmax_K(W @ desc + b)            [B, K, N]
  agg[b,d,k] = sum_n scores[b,k,n] desc[b,d,n]
  vlad = agg - centers * sum_n(scores);  intra-L2-norm over D; global L2.

Sharding: data-parallel over B across 8 cores (2 items per core);
W/b/centers replicated.

What it does beyond the 58.6us baseline (each validated by per-packet
trace analysis):
  - deep prefetch: all 16 desc-stream DMAs (8x 512KB da on the sync
    ring, 8x 540KB dt on the gpsimd ring) are issued up front and the
    SBUF pools hold the whole 8.7 MB/core working set, so both HWDGE
    rings keep 3-4 descriptors live (per-queue throughput scales with
    that) and stream at a combined ~340-410 GB/s.
  - the scalar ring (whose engine runs the softmax Exps) carries only
    the three tiny consts; putting stream descriptors there
    head-of-line blocks the Exps for ~10us (v2 lesson).
  - ssum is folded into the aggregation matmul: the host appends a
    ones-column to the n-major desc layout (row pitch 528 keeps the
    DoubleRow APs 16B-aligned) and agg accumulates into two PSUM banks
    per item ([K,256] + [K,257]); ssum is the last column. This kills
    32 tiny ssum matmuls + their LDWEIGHTS per core.
  - the final normalization (sum-sq over D, rsqrt, x0.125) moved to
    the host (~1.6 MFLOP); the device tail is two
    scalar_tensor_tensor ops + output DMAs, eliminating the ACT
    Ln/Exp table reloads (1.28us each) that serialized the v1 tail.
  - ~40 tiny warmup matmuls nudge the PE HAM clock gate toward 8/8
    before the first real matmul.
  - drain order: the last group's transposes run before the
    second-to-last group's aggregation matmuls, so the final softmax
    DVE chain hides under PE work.
"""

import sys

sys.path.insert(0, "/opt/trn_rl_repo")

import numpy as np
import ml_dtypes

B, D, K, N = 16, 512, 64, 4096
N_CORES = 8
B_PER = B // N_CORES           # 2 items per core
DT = D // 128                  # 4 d-tiles
NS = 4                         # strips per item (1024 n each)
CH = 8                         # 128-col n-chunks per strip
DTP = 528                      # dt row pitch: 512 d + ones col + pad

_CACHE = {}


def _build():
    import concourse.bass as bass  # noqa: F401
    import concourse.tile as tile
    from concourse import bacc, mybir
    from contextlib import ExitStack

    bf16 = mybir.dt.bfloat16
    f8 = mybir.dt.float8e4
    f32 = mybir.dt.float32
    AF = mybir.ActivationFunctionType
    OP = mybir.AluOpType
    AX = mybir.AxisListType
    DR = mybir.MatmulPerfMode.DoubleRow

    nc = bacc.Bacc("TRN2", target_bir_lowering=False, debug=False,
                   num_devices=N_CORES)

    da_d = nc.dram_tensor("da", [B_PER, NS, 128, DT, 1024], f8,
                          kind="ExternalInput").ap()
    da0_d = nc.dram_tensor("da0", [B_PER, 2, 128, 2, 1024], f8,
                           kind="ExternalInput").ap()
    dt_d = nc.dram_tensor("dt", [B_PER, NS, 128, CH, DTP], f8,
                          kind="ExternalInput").ap()
    wt_d = nc.dram_tensor("wt", [128, DT, K], f8, kind="ExternalInput").ap()
    eye_d = nc.dram_tensor("eye", [64, 64], bf16,
                           kind="ExternalInput").ap()
    bvec_d = nc.dram_tensor("bvec", [K, 1], f32, kind="ExternalInput").ap()
    cnegb_d = nc.dram_tensor("cnegb", [K, D], f32,
                             kind="ExternalInput").ap()
    out_d = nc.dram_tensor("out", [B_PER, K, D], bf16,
                           kind="ExternalOutput").ap()

    with tile.TileContext(nc) as tc, ExitStack() as ctx:
        const = ctx.enter_context(tc.tile_pool(name="const", bufs=1))
        sdesc = ctx.enter_context(tc.tile_pool(name="sdesc", bufs=8))
        sdt = ctx.enter_context(tc.tile_pool(name="sdt", bufs=8))
        pexp = ctx.enter_context(tc.tile_pool(name="pexp", bufs=4))
        psoft = ctx.enter_context(tc.tile_pool(name="psoft", bufs=4))
        small = ctx.enter_context(tc.tile_pool(name="small", bufs=16))
        med = ctx.enter_context(tc.tile_pool(name="med", bufs=2))
        # PSUM bank budget (8): sc 2 + xt 2 + agg 4 (A/B per item)
        ps_sc = ctx.enter_context(tc.tile_pool(name="ps_sc", bufs=2,
                                               space="PSUM"))
        ps_xt = ctx.enter_context(tc.tile_pool(name="ps_xt", bufs=2,
                                               space="PSUM"))
        ps_agg = ctx.enter_context(tc.tile_pool(name="ps_agg", bufs=4,
                                                space="PSUM"))

        # ---- constants: tiny, at the head of the gpsimd ring. The dt
        # stream behind them has ~4.6us of slack before its first
        # consumer (mm2 lags mm1 by 2 groups), so the tiny-packet
        # poisoning lands on slack — while wt arrives by ~9us instead
        # of ~15us on a starved ring, unblocking the first mm1. ----
        bvec_sb = const.tile([K, 1], f32, tag="bvec")
        nc.gpsimd.dma_start(out=bvec_sb[:], in_=bvec_d[:])
        eye_sb = const.tile([64, 64], bf16, tag="eye")
        nc.gpsimd.dma_start(out=eye_sb[:], in_=eye_d[:])
        wt_sb = const.tile([128, DT, K], f8, tag="wt")
        nc.gpsimd.dma_start(out=wt_sb[:], in_=wt_d[:])
        b_sb = bvec_sb[:]
        wsrc_sb = const.tile([128, 64], f8, tag="wsrc")
        nc.vector.memset(wsrc_sb[:], 1.0)

        # ---- issue the full desc stream up front (never back-pressured:
        # the pools hold all 8 tiles per stream) ----
        dbf = [[None] * NS for _ in range(B_PER)]
        dtt = [[None] * NS for _ in range(B_PER)]
        for s in range(NS):
            for i in range(B_PER):
                dbf[i][s] = sdesc.tile([128, DT, 1024], f8, tag="dbf",
                                       name=f"dbf{i}_{s}")
                if s == 0:
                    # first strip's blocks split in two so mm1 T=0
                    # starts on the first half ~1.5us sooner
                    nc.sync.dma_start(out=dbf[i][s][:, 0:2, :],
                                      in_=da0_d[i, 0])
                    nc.sync.dma_start(out=dbf[i][s][:, 2:4, :],
                                      in_=da0_d[i, 1])
                else:
                    nc.sync.dma_start(out=dbf[i][s][:], in_=da_d[i, s])
                dtt[i][s] = sdt.tile([128, CH, DTP], f8, tag="dT",
                                     name=f"dT{i}_{s}")
                nc.gpsimd.dma_start(out=dtt[i][s][:], in_=dt_d[i, s])
        cnegb_sb = const.tile([K, D], f32, tag="cnegb")
        nc.gpsimd.dma_start(out=cnegb_sb[:], in_=cnegb_d[:])
        cneg_sb = cnegb_sb[:]

        # ---- HAM warmup: back-to-back tiny matmuls ----
        warm_ps = ps_sc.tile([64, 512], f32, tag="sc", name="warm")
        for _ in range(40):
            nc.tensor.matmul(warm_ps[:, 0:64], lhsT=wsrc_sb[:],
                             rhs=wsrc_sb[:], start=True, stop=True)

        aggA = [ps_agg.tile([64, 512], f32, tag="agg", name=f"aggA{i}")
                for i in range(B_PER)]
        aggB = [ps_agg.tile([64, 512], f32, tag="agg", name=f"aggB{i}")
                for i in range(B_PER)]

        pend_tr = []   # (i, s, [(u, exp)])
        pend_mm2 = []  # (i, s, soft_g)

        def emit_tr(grp):
            i, s, pair = grp
            xt = ps_xt.tile([128, CH, K], bf16, tag="xt",
                            name=f"xt{i}_{s}")
            for u, exp_h in pair:
                for cc in range(4):
                    nc.tensor.transpose(
                        xt[:, 4 * u + cc, :],
                        exp_h[:, 128 * cc:128 * (cc + 1)],
                        eye_sb[:],
                    )
            z8 = small.tile([128, CH], f32, tag="z", name=f"z{i}_{s}")
            nc.vector.reduce_sum(z8[:], xt[:], axis=AX.X)
            r8 = small.tile([128, CH], f32, tag="r", name=f"r{i}_{s}")
            nc.vector.reciprocal(r8[:], z8[:])
            soft_g = psoft.tile([128, CH, K], f8, tag="soft",
                                name=f"soft{i}_{s}")
            nc.vector.tensor_mul(
                soft_g[:], xt[:],
                r8[:, :, None].broadcast_to((128, CH, K)))
            pend_mm2.append((i, s, soft_g))

        def emit_mm2(grp):
            i, s, soft_g = grp
            dt_t = dtt[i][s]
            for p in range(CH // 2):
                lhsT = soft_g[:, 2 * p:2 * p + 2, :]
                st = (s == 0 and p == 0)
                sp = (s == NS - 1 and p == CH // 2 - 1)
                nc.tensor.matmul(
                    aggA[i][:, 0:256], lhsT=lhsT,
                    rhs=dt_t[:, 2 * p:2 * p + 2, 0:256],
                    perf_mode=DR, start=st, stop=sp)
                nc.tensor.matmul(
                    aggB[i][:, 0:257], lhsT=lhsT,
                    rhs=dt_t[:, 2 * p:2 * p + 2, 256:513],
                    perf_mode=DR, start=st, stop=sp)

        def emit_tail(i):
            # vlad = cneg * ssum + agg; ssum is the ones-column of aggB.
            # Final intra/global L2 normalization happens on the host.
            ss = aggB[i][:, 256:257]
            vlad_sb = med.tile([K, D], bf16, tag="vlad", name=f"vlad{i}")
            nc.vector.scalar_tensor_tensor(
                vlad_sb[:, 0:256], in0=cneg_sb[:, 0:256], scalar=ss,
                in1=aggA[i][:, 0:256], op0=OP.mult, op1=OP.add)
            nc.vector.scalar_tensor_tensor(
                vlad_sb[:, 256:512], in0=cneg_sb[:, 256:512], scalar=ss,
                in1=aggB[i][:, 0:256], op0=OP.mult, op1=OP.add)
            nc.sync.dma_start(out=out_d[i], in_=vlad_sb[:])

        for s in range(NS):
            for i in range(B_PER):
                # emit ready transpose/mm2 work BEFORE this group's mm1:
                # the PE queue is in-order, so a data-gated mm1 ahead of
                # ready work would idle the engine. Lags stay tr=1,
                # mm2=2 groups.
                if pend_tr:
                    emit_tr(pend_tr.pop(0))
                if len(pend_mm2) > 1:
                    emit_mm2(pend_mm2.pop(0))
                pair = []
                for u in range(2):
                    # mm1: scores [64k, 512n], fp8 DoubleRow, W stationary
                    scp = ps_sc.tile([64, 512], f32, tag="sc",
                                     name=f"sc{i}_{s}_{u}")
                    for T in range(2):
                        nc.tensor.matmul(
                            scp[:],
                            lhsT=wt_sb[:, 2 * T:2 * T + 2, :],
                            rhs=dbf[i][s][:, 2 * T:2 * T + 2,
                                          512 * u:512 * (u + 1)],
                            perf_mode=DR, start=(T == 0), stop=(T == 1))
                    exp_h = pexp.tile([64, 512], bf16, tag="exps",
                                      name=f"exps{i}_{s}_{u}")
                    nc.scalar.activation(out=exp_h[:], in_=scp[:],
                                         func=AF.Exp, bias=b_sb,
                                         scale=1.0)
                    pair.append((u, exp_h))
                pend_tr.append((i, s, pair))
        # drain: last group's transposes first so its softmax DVE chain
        # hides under the second-to-last group's aggregation matmuls
        while pend_tr:
            emit_tr(pend_tr.pop(0))
        emit_mm2(pend_mm2.pop(0))   # (s3, i0): finalizes item 0's agg
        emit_tail(0)                # overlaps the last mm2 group on PE
        emit_mm2(pend_mm2.pop(0))   # (s3, i1)
        emit_tail(1)

    nc.compile()
    return nc


def _get_nc():
    if "nc" not in _CACHE:
        _CACHE["nc"] = _build()
    return _CACHE["nc"]


def _host_inputs(descriptors, W, b, centers):
    f8 = ml_dtypes.float8_e4m3fn
    d16 = np.asarray(descriptors, dtype=np.float32).astype(f8)  # [B, D, N]
    wt = np.ascontiguousarray(
        W.astype(np.float32).T.reshape(DT, 128, K).transpose(1, 0, 2)
    ).astype(f8)                                       # [128, DT, K] p-major
    eye = np.eye(64, dtype=np.float32).astype(ml_dtypes.bfloat16)
    bvec = np.ascontiguousarray(b.astype(np.float32).reshape(K, 1))
    cnegb = np.ascontiguousarray(-centers.astype(np.float32).T)  # [K, D]
    common = {"wt": wt, "eye": eye, "bvec": bvec, "cnegb": cnegb}
    in_maps = []
    for core in range(N_CORES):
        dc = d16[B_PER * core:B_PER * (core + 1)]        # [2, D, N] fp8
        # da[i, s, p, t, x] = desc[i, 128t+p, 1024s+x]
        da = dc.reshape(B_PER, DT, 128, NS, 1024
                        ).transpose(0, 3, 2, 1, 4)
        # first strip (s=0) again as contiguous halves per item
        da0 = dc.reshape(B_PER, 2, 2, 128, NS, 1024
                         )[:, :, :, :, 0].transpose(0, 1, 3, 2, 4)
        # dt[i, s, p, c, d] = desc[i, d, 1024s+128c+p]; col 512 = 1.0
        dt_ = np.zeros((B_PER, NS, 128, CH, DTP), dtype=f8)
        dt_[..., 0:512] = dc.reshape(B_PER, D, NS, CH, 128
                                     ).transpose(0, 2, 4, 3, 1)
        dt_[..., 512] = 1.0
        m = dict(common)
        m["da"] = np.ascontiguousarray(da)
        m["da0"] = np.ascontiguousarray(da0)
        m["dt"] = dt_
        in_maps.append(m)
    return in_maps


def _run(inputs, trace=False):
    from concourse.bass_utils import run_bass_kernel_spmd

    descriptors = np.asarray(inputs["descriptors"])
    W = np.asarray(inputs["W"])
    b = np.asarray(inputs["b"])
    centers = np.asarray(inputs["centers"])
    nc = _get_nc()
    in_maps = _host_inputs(descriptors, W, b, centers)
    res = run_bass_kernel_spmd(nc, in_maps, list(range(N_CORES)), trace=trace)
    outs = []
    for core in range(N_CORES):
        o = np.asarray(res.results[core]["out"], dtype=np.float32)
        # intra-normalize over D per (item, k), then global L2 = 1/sqrt(K)
        nrm = np.sqrt(np.sum(o * o, axis=2, keepdims=True))
        o = o / np.maximum(nrm, 1e-20) * (1.0 / np.sqrt(K))
        outs.append(np.transpose(o, (0, 2, 1)).reshape(B_PER, D * K))
    full = np.concatenate(outs, axis=0).astype(np.float32)
    return full, res


def kernel(**inputs):
    out, _ = _run(inputs, trace=False)
    return out


if __name__ == "__main__":
    rng = np.random.default_rng(0)
    inputs = {
        "descriptors": rng.standard_normal((B, D, N), dtype=np.float32),
        "W": (rng.standard_normal((K, D)) * 0.05).astype(np.float32),
        "b": (rng.standard_normal((K,)) * 0.05).astype(np.float32),
        "centers": rng.standard_normal((D, K)).astype(np.float32),
    }
    out = kernel(**inputs)
    print("out shape:", out.shape, out.dtype)


# revision 39
# speedup vs baseline: 1.0843x; 1.0843x over previous
"""NetVLAD layer on 8 Trainium2 NeuronCores (Bass/Tile), v3-final.

Problem: descriptors [B=16, D=512, N=4096] f32, W [K=64, D], b [K],
centers [D, K].
  scores = softmax_K(W @ desc + b)            [B, K, N]
  agg[b,d,k] = sum_n scores[b,k,n] desc[b,d,n]
  vlad = agg - centers * sum_n(scores);  intra-L2-norm over D; global L2.

Sharding: data-parallel over B across 8 cores (2 items per core);
W/b/centers replicated.

What it does beyond the 58.6us baseline (each validated by per-packet
trace analysis):
  - deep prefetch: all 16 desc-stream DMAs (8x 512KB da on the sync
    ring, 8x 540KB dt on the gpsimd ring) are issued up front and the
    SBUF pools hold the whole 8.7 MB/core working set, so both HWDGE
    rings keep 3-4 descriptors live (per-queue throughput scales with
    that) and stream at a combined ~340-410 GB/s.
  - the scalar ring (whose engine runs the softmax Exps) carries only
    the three tiny consts; putting stream descriptors there
    head-of-line blocks the Exps for ~10us (v2 lesson).
  - ssum is folded into the aggregation matmul: the host appends a
    ones-column to the n-major desc layout (row pitch 528 keeps the
    DoubleRow APs 16B-aligned) and agg accumulates into two PSUM banks
    per item ([K,256] + [K,257]); ssum is the last column. This kills
    32 tiny ssum matmuls + their LDWEIGHTS per core.
  - the final normalization (sum-sq over D, rsqrt, x0.125) moved to
    the host (~1.6 MFLOP); the device tail is two
    scalar_tensor_tensor ops + output DMAs, eliminating the ACT
    Ln/Exp table reloads (1.28us each) that serialized the v1 tail.
  - ~40 tiny warmup matmuls nudge the PE HAM clock gate toward 8/8
    before the first real matmul.
  - drain order: the last group's transposes run before the
    second-to-last group's aggregation matmuls, so the final softmax
    DVE chain hides under PE work.
"""

import sys

sys.path.insert(0, "/opt/trn_rl_repo")

import numpy as np
import ml_dtypes

B, D, K, N = 16, 512, 64, 4096
N_CORES = 8
B_PER = B // N_CORES           # 2 items per core
DT = D // 128                  # 4 d-tiles
NS = 4                         # strips per item (1024 n each)
CH = 8                         # 128-col n-chunks per strip
DTP = 528                      # dt row pitch: 512 d + ones col + pad

_CACHE = {}


def _build():
    import concourse.bass as bass  # noqa: F401
    import concourse.tile as tile
    from concourse import bacc, mybir
    from contextlib import ExitStack

    bf16 = mybir.dt.bfloat16
    f8 = mybir.dt.float8e4
    f32 = mybir.dt.float32
    AF = mybir.ActivationFunctionType
    OP = mybir.AluOpType
    AX = mybir.AxisListType
    DR = mybir.MatmulPerfMode.DoubleRow

    nc = bacc.Bacc("TRN2", target_bir_lowering=False, debug=False,
                   num_devices=N_CORES)

    da_d = nc.dram_tensor("da", [B_PER, NS, 128, DT, 1024], f8,
                          kind="ExternalInput").ap()
    da0_d = nc.dram_tensor("da0", [2, 128, 2, 1024], f8,
                           kind="ExternalInput").ap()
    dt_d = nc.dram_tensor("dt", [B_PER, NS, 128, CH, DTP], f8,
                          kind="ExternalInput").ap()
    wt_d = nc.dram_tensor("wt", [128, DT, K], f8, kind="ExternalInput").ap()
    eye_d = nc.dram_tensor("eye", [64, 64], bf16,
                           kind="ExternalInput").ap()
    bvec_d = nc.dram_tensor("bvec", [K, 1], f32, kind="ExternalInput").ap()
    cnegb_d = nc.dram_tensor("cnegb", [K, D], f32,
                             kind="ExternalInput").ap()
    out_d = nc.dram_tensor("out", [B_PER, K, D], bf16,
                           kind="ExternalOutput").ap()

    with tile.TileContext(nc) as tc, ExitStack() as ctx:
        const = ctx.enter_context(tc.tile_pool(name="const", bufs=1))
        sdesc = ctx.enter_context(tc.tile_pool(name="sdesc", bufs=8))
        sdt = ctx.enter_context(tc.tile_pool(name="sdt", bufs=8))
        pexp = ctx.enter_context(tc.tile_pool(name="pexp", bufs=4))
        psoft = ctx.enter_context(tc.tile_pool(name="psoft", bufs=4))
        small = ctx.enter_context(tc.tile_pool(name="small", bufs=16))
        med = ctx.enter_context(tc.tile_pool(name="med", bufs=2))
        # PSUM bank budget (8): sc 2 + xt 2 + agg 4 (A/B per item)
        ps_sc = ctx.enter_context(tc.tile_pool(name="ps_sc", bufs=2,
                                               space="PSUM"))
        ps_xt = ctx.enter_context(tc.tile_pool(name="ps_xt", bufs=2,
                                               space="PSUM"))
        ps_agg = ctx.enter_context(tc.tile_pool(name="ps_agg", bufs=4,
                                                space="PSUM"))

        # ---- constants: tiny, at the head of the gpsimd ring. The dt
        # stream behind them has ~4.6us of slack before its first
        # consumer (mm2 lags mm1 by 2 groups), so the tiny-packet
        # poisoning lands on slack — while wt arrives by ~9us instead
        # of ~15us on a starved ring, unblocking the first mm1. ----
        bvec_sb = const.tile([K, 1], f32, tag="bvec")
        nc.gpsimd.dma_start(out=bvec_sb[:], in_=bvec_d[:])
        eye_sb = const.tile([64, 64], bf16, tag="eye")
        nc.gpsimd.dma_start(out=eye_sb[:], in_=eye_d[:])
        wt_sb = const.tile([128, DT, K], f8, tag="wt")
        nc.gpsimd.dma_start(out=wt_sb[:], in_=wt_d[:])
        b_sb = bvec_sb[:]
        wsrc_sb = const.tile([128, 64], f8, tag="wsrc")
        nc.vector.memset(wsrc_sb[:], 1.0)

        # ---- issue the full desc stream up front (never back-pressured:
        # the pools hold all 8 tiles per stream) ----
        dbf = [[None] * NS for _ in range(B_PER)]
        dtt = [[None] * NS for _ in range(B_PER)]
        for s in range(NS):
            for i in range(B_PER):
                dbf[i][s] = sdesc.tile([128, DT, 1024], f8, tag="dbf",
                                       name=f"dbf{i}_{s}")
                if s == 0 and i == 0:
                    # first block split in two so mm1 T=0 starts on the
                    # first half ~1.5us sooner
                    nc.sync.dma_start(out=dbf[i][s][:, 0:2, :],
                                      in_=da0_d[0])
                    nc.sync.dma_start(out=dbf[i][s][:, 2:4, :],
                                      in_=da0_d[1])
                else:
                    nc.sync.dma_start(out=dbf[i][s][:], in_=da_d[i, s])
                dtt[i][s] = sdt.tile([128, CH, DTP], f8, tag="dT",
                                     name=f"dT{i}_{s}")
                nc.gpsimd.dma_start(out=dtt[i][s][:], in_=dt_d[i, s])
        cnegb_sb = const.tile([K, D], f32, tag="cnegb")
        nc.gpsimd.dma_start(out=cnegb_sb[:], in_=cnegb_d[:])
        cneg_sb = cnegb_sb[:]

        # ---- HAM warmup: back-to-back tiny matmuls ----
        warm_ps = ps_sc.tile([64, 512], f32, tag="sc", name="warm")
        for _ in range(40):
            nc.tensor.matmul(warm_ps[:, 0:64], lhsT=wsrc_sb[:],
                             rhs=wsrc_sb[:], start=True, stop=True)

        aggA = [ps_agg.tile([64, 512], f32, tag="agg", name=f"aggA{i}")
                for i in range(B_PER)]
        aggB = [ps_agg.tile([64, 512], f32, tag="agg", name=f"aggB{i}")
                for i in range(B_PER)]

        pend_tr = []   # (i, s, [(u, exp)])
        pend_mm2 = []  # (i, s, soft_g)

        def emit_tr(grp):
            i, s, pair = grp
            xt = ps_xt.tile([128, CH, K], bf16, tag="xt",
                            name=f"xt{i}_{s}")
            for u, exp_h in pair:
                for cc in range(4):
                    nc.tensor.transpose(
                        xt[:, 4 * u + cc, :],
                        exp_h[:, 128 * cc:128 * (cc + 1)],
                        eye_sb[:],
                    )
            z8 = small.tile([128, CH], f32, tag="z", name=f"z{i}_{s}")
            nc.vector.reduce_sum(z8[:], xt[:], axis=AX.X)
            r8 = small.tile([128, CH], f32, tag="r", name=f"r{i}_{s}")
            nc.vector.reciprocal(r8[:], z8[:])
            soft_g = psoft.tile([128, CH, K], f8, tag="soft",
                                name=f"soft{i}_{s}")
            nc.vector.tensor_mul(
                soft_g[:], xt[:],
                r8[:, :, None].broadcast_to((128, CH, K)))
            pend_mm2.append((i, s, soft_g))

        def emit_mm2(grp):
            i, s, soft_g = grp
            dt_t = dtt[i][s]
            for p in range(CH // 2):
                lhsT = soft_g[:, 2 * p:2 * p + 2, :]
                st = (s == 0 and p == 0)
                sp = (s == NS - 1 and p == CH // 2 - 1)
                nc.tensor.matmul(
                    aggA[i][:, 0:256], lhsT=lhsT,
                    rhs=dt_t[:, 2 * p:2 * p + 2, 0:256],
                    perf_mode=DR, start=st, stop=sp)
                nc.tensor.matmul(
                    aggB[i][:, 0:257], lhsT=lhsT,
                    rhs=dt_t[:, 2 * p:2 * p + 2, 256:513],
                    perf_mode=DR, start=st, stop=sp)

        def emit_tail(i):
            # vlad = cneg * ssum + agg; ssum is the ones-column of aggB.
            # Final intra/global L2 normalization happens on the host.
            ss = aggB[i][:, 256:257]
            vlad_sb = med.tile([K, D], bf16, tag="vlad", name=f"vlad{i}")
            nc.vector.scalar_tensor_tensor(
                vlad_sb[:, 0:256], in0=cneg_sb[:, 0:256], scalar=ss,
                in1=aggA[i][:, 0:256], op0=OP.mult, op1=OP.add)
            nc.vector.scalar_tensor_tensor(
                vlad_sb[:, 256:512], in0=cneg_sb[:, 256:512], scalar=ss,
                in1=aggB[i][:, 0:256], op0=OP.mult, op1=OP.add)
            nc.sync.dma_start(out=out_d[i], in_=vlad_sb[:])

        for s in range(NS):
            for i in range(B_PER):
                # emit ready transpose/mm2 work BEFORE this group's mm1:
                # the PE queue is in-order, so a data-gated mm1 ahead of
                # ready work would idle the engine. Lags stay tr=1,
                # mm2=2 groups.
                if pend_tr:
                    emit_tr(pend_tr.pop(0))
                if len(pend_mm2) > 1:
                    emit_mm2(pend_mm2.pop(0))
                pair = []
                for u in range(2):
                    # mm1: scores [64k, 512n], fp8 DoubleRow, W stationary
                    scp = ps_sc.tile([64, 512], f32, tag="sc",
                                     name=f"sc{i}_{s}_{u}")
                    for T in range(2):
                        nc.tensor.matmul(
                            scp[:],
                            lhsT=wt_sb[:, 2 * T:2 * T + 2, :],
                            rhs=dbf[i][s][:, 2 * T:2 * T + 2,
                                          512 * u:512 * (u + 1)],
                            perf_mode=DR, start=(T == 0), stop=(T == 1))
                    exp_h = pexp.tile([64, 512], bf16, tag="exps",
                                      name=f"exps{i}_{s}_{u}")
                    nc.scalar.activation(out=exp_h[:], in_=scp[:],
                                         func=AF.Exp, bias=b_sb,
                                         scale=1.0)
                    pair.append((u, exp_h))
                pend_tr.append((i, s, pair))
        # drain: last group's transposes first so its softmax DVE chain
        # hides under the second-to-last group's aggregation matmuls
        while pend_tr:
            emit_tr(pend_tr.pop(0))
        emit_mm2(pend_mm2.pop(0))   # (s3, i0): finalizes item 0's agg
        emit_tail(0)                # overlaps the last mm2 group on PE
        emit_mm2(pend_mm2.pop(0))   # (s3, i1)
        emit_tail(1)

    nc.compile()
    return nc


def _get_nc():
    if "nc" not in _CACHE:
        _CACHE["nc"] = _build()
    return _CACHE["nc"]


def _host_inputs(descriptors, W, b, centers):
    f8 = ml_dtypes.float8_e4m3fn
    d16 = np.asarray(descriptors, dtype=np.float32).astype(f8)  # [B, D, N]
    wt = np.ascontiguousarray(
        W.astype(np.float32).T.reshape(DT, 128, K).transpose(1, 0, 2)
    ).astype(f8)                                       # [128, DT, K] p-major
    eye = np.eye(64, dtype=np.float32).astype(ml_dtypes.bfloat16)
    bvec = np.ascontiguousarray(b.astype(np.float32).reshape(K, 1))
    cnegb = np.ascontiguousarray(-centers.astype(np.float32).T)  # [K, D]
    common = {"wt": wt, "eye": eye, "bvec": bvec, "cnegb": cnegb}
    in_maps = []
    for core in range(N_CORES):
        dc = d16[B_PER * core:B_PER * (core + 1)]        # [2, D, N] fp8
        # da[i, s, p, t, x] = desc[i, 128t+p, 1024s+x]
        da = dc.reshape(B_PER, DT, 128, NS, 1024
                        ).transpose(0, 3, 2, 1, 4)
        # first block (i=0, s=0) again as two contiguous halves
        da0 = dc[0:1].reshape(1, 2, 2, 128, NS, 1024
                              )[:, :, :, :, 0].transpose(0, 1, 3, 2, 4)[0]
        # dt[i, s, p, c, d] = desc[i, d, 1024s+128c+p]; col 512 = 1.0
        dt_ = np.zeros((B_PER, NS, 128, CH, DTP), dtype=f8)
        dt_[..., 0:512] = dc.reshape(B_PER, D, NS, CH, 128
                                     ).transpose(0, 2, 4, 3, 1)
        dt_[..., 512] = 1.0
        m = dict(common)
        m["da"] = np.ascontiguousarray(da)
        m["da0"] = np.ascontiguousarray(da0)
        m["dt"] = dt_
        in_maps.append(m)
    return in_maps


def _run(inputs, trace=False):
    from concourse.bass_utils import run_bass_kernel_spmd

    descriptors = np.asarray(inputs["descriptors"])
    W = np.asarray(inputs["W"])
    b = np.asarray(inputs["b"])
    centers = np.asarray(inputs["centers"])
    nc = _get_nc()
    in_maps = _host_inputs(descriptors, W, b, centers)
    res = run_bass_kernel_spmd(nc, in_maps, list(range(N_CORES)), trace=trace)
    outs = []
    for core in range(N_CORES):
        o = np.asarray(res.results[core]["out"], dtype=np.float32)
        # intra-normalize over D per (item, k), then global L2 = 1/sqrt(K)
        nrm = np.sqrt(np.sum(o * o, axis=2, keepdims=True))
        o = o / np.maximum(nrm, 1e-20) * (1.0 / np.sqrt(K))
        outs.append(np.transpose(o, (0, 2, 1)).reshape(B_PER, D * K))
    full = np.concatenate(outs, axis=0).astype(np.float32)
    return full, res


def kernel(**inputs):
    out, _ = _run(inputs, trace=False)
    return out


if __name__ == "__main__":
    rng = np.random.default_rng(0)
    inputs = {
        "descriptors": rng.standard_normal((B, D, N), dtype=np.float32),
        "W": (rng.standard_normal((K, D)) * 0.05).astype(np.float32),
        "b": (rng.standard_normal((K,)) * 0.05).astype(np.float32),
        "centers": rng.standard_normal((D, K)).astype(np.float32),
    }
    out = kernel(**inputs)
    print("out shape:", out.shape, out.dtype)
